# revision 1
# baseline (speedup 1.0000x reference)
"""Trainium2 Bass kernel for nn_Head (single-head causal attention).

Contract: kernel(**inputs) takes FULL inputs (x [8,2048,1024] f32,
Wk/Wq/Wv [64,1024] f32) and returns the FULL output [8,2048,64] f32.
Data-parallel over batch B=8 across the 8 NeuronCores (one batch row per
core); each core runs an identical single-core program.

Host-side prep (inside kernel(), pure numpy marshaling):
  - xT = x[b].T               -> projections need c-on-partitions; doing the
                                 transpose on host avoids any on-chip
                                 transpose of the 8MB activation tensor.
  - wkq = concat([Wk/32, Wq]).T  (fold 1/sqrt(C)=1/32 into Wk so scores come
                                 out pre-scaled; fused so the kq projection
                                 matmul uses the full 128-wide PE array)
  - wv  = Wv.T

Device kernel (per core), all f32:
  kqT = wkq.T @ xT            [128, 2048] PSUM->SBUF (rows 0:64 = kT scaled,
                                                      rows 64:128 = qT)
  vT  = wv.T @ xT             [64, 2048], then PE-transpose to v [2048, 64],
                              augmented with a ones column -> v_aug [128,16,65]
  For each s-tile i (128 rows of ST = wei^T):
    ST[s, t] = qT[:, s-tile].T @ kT   (t >= 128*i only: causal skip)
    PT = exp(ST)  (safe without max-subtraction: |S| < 0.75 for this problem)
    diagonal 128x128 block multiplied by upper-triangular 0/1 mask
    OT[j] += v_aug[i].T @ PT          [65, 512] PSUM accumulators, row 64 is
                                      the softmax denominator (ones column)
  Finally PE-transpose OT -> [128, 65] tiles, normalize rows by col 64, DMA out.
"""

import sys

if "/opt/trn_rl_repo" not in sys.path:
    sys.path.insert(0, "/opt/trn_rl_repo")

import numpy as np

B = 8
T = 2048
C = 1024
H = 64
P = 128
CB = C // P        # 8 contraction chunks
TJ = T // 512      # 4 column chunks of 512
NT = T // P        # 16 s-tiles
N_CORES = 8

_NC_CACHE = {}


def _build_nc():
    import concourse.bass as bass
    import concourse.mybir as mybir
    import concourse.tile as tile
    from concourse.bass import ts
    from concourse.masks import make_identity, make_upper_triangular

    fp32 = mybir.dt.float32
    bf16 = mybir.dt.bfloat16
    EXP = mybir.ActivationFunctionType.Exp

    nc = bass.Bass(target_bir_lowering=False, debug=False)
    xt_d = nc.declare_dram_parameter("xt", [C, T], bf16, isOutput=False)
    wkq_d = nc.declare_dram_parameter("wkq", [C, P], bf16, isOutput=False)
    wv_d = nc.declare_dram_parameter("wv", [C, H], bf16, isOutput=False)
    out_d = nc.declare_dram_parameter("out", [T, H], fp32, isOutput=True)

    from contextlib import ExitStack

    with tile.TileContext(nc) as tc, ExitStack() as stk:
        pers = stk.enter_context(tc.tile_pool(name="pers", bufs=1))
        xt_sb = pers.tile([P, CB, T], bf16, tag="xt_sb", name="xt_sb")        # xT bands: band cb = xT[128cb:128cb+128, :]
        wkq_sb = pers.tile([P, CB, P], bf16, tag="wkq_sb", name="wkq_sb")
        wv_sb = pers.tile([P, CB, H], bf16, tag="wv_sb", name="wv_sb")
        kt_sb = pers.tile([H, T], bf16, tag="kt_sb", name="kt_sb")
        qt_sb = pers.tile([H, T], bf16, tag="qt_sb", name="qt_sb")
        vt_sb = pers.tile([H, T], bf16, tag="vt_sb", name="vt_sb")
        vaug_sb = pers.tile([P, NT, H + 1], bf16, tag="vaug_sb", name="vaug_sb")
        ot_sb = pers.tile([H + 1, T], fp32, tag="ot_sb", name="ot_sb")
        o_sb = pers.tile([P, NT, H], fp32, tag="o_sb", name="o_sb")
        ident = pers.tile([P, P], fp32, tag="ident", name="ident")
        identb = pers.tile([H, H], bf16, tag="identb", name="identb")
        tri = pers.tile([P, P], bf16, tag="tri", name="tri")
        rec_sb = pers.tile([P, NT], fp32, tag="rec_sb", name="rec_sb")

        make_identity(nc, ident[:])
        make_identity(nc, identb[:])
        # ST block [s_local, t_local]: keep s <= t -> upper triangular incl diagonal
        make_upper_triangular(nc, tri[:], val=1.0, diag=True)

        nc.sync.dma_start(wkq_sb[:], wkq_d.rearrange("(o p) m -> p o m", p=P))
        nc.sync.dma_start(wv_sb[:], wv_d.rearrange("(o p) m -> p o m", p=P))
        for cb in range(CB):
            nc.sync.dma_start(xt_sb[:, cb, :], xt_d[cb * P : (cb + 1) * P, :])

        # ---- projections: kqT [128, T] then vT [64, T] ----
        # One pool holding both tag groups (8 banks live) so the v matmuls
        # don't WAR-serialize behind the kq PSUM->SBUF copies: the PE rolls
        # kq mms -> v mms while the DVE drains the kq copies in parallel.
        with tc.tile_pool(name="pp", bufs=4, space="PSUM") as pp:
            kq_ps = [pp.tile([P, 512], fp32, tag="kq", name=f"kq{j}") for j in range(TJ)]
            v_ps = [pp.tile([H, 512], fp32, tag="v", name=f"v{j}") for j in range(TJ)]
            for cb in range(CB):
                for j in range(TJ):
                    nc.tensor.matmul(
                        kq_ps[j], wkq_sb[:, cb, :], xt_sb[:, cb, ts(j, 512)],
                        start=(cb == 0), stop=(cb == CB - 1),
                    )
            for j in range(TJ):
                nc.vector.tensor_copy(kt_sb[:, ts(j, 512)], kq_ps[j][0:H, :])
                nc.vector.tensor_copy(qt_sb[:, ts(j, 512)], kq_ps[j][H:P, :])
            for cb in range(CB):
                for j in range(TJ):
                    nc.tensor.matmul(
                        v_ps[j], wv_sb[:, cb, :], xt_sb[:, cb, ts(j, 512)],
                        start=(cb == 0), stop=(cb == CB - 1),
                    )
            for j in range(TJ):
                nc.vector.tensor_copy(vt_sb[:, ts(j, 512)], v_ps[j][:, :])

        # ---- v natural [s, d] + ones column ----
        nc.any.memset(vaug_sb[:, :, H], 1.0)
        with tc.tile_pool(name="vtp", bufs=2, space="PSUM") as vtp:
            for i in range(NT):
                vps = vtp.tile([P, H], bf16, tag="vt", name=f"vt{i}")
                nc.tensor.transpose(vps, vt_sb[:, ts(i, P)], identb[:])
                nc.vector.tensor_copy(vaug_sb[:, i, 0:H], vps)

        # ---- attention: 1024-wide ST tiles, one exp per tile, PVs lag STs
        # by one s-tile so the PE FIFO never stalls waiting on the exp of the
        # tile it just produced ----
        with (
            tc.tile_pool(name="stp", bufs=2, space="PSUM") as stp,
            tc.tile_pool(name="otp", bufs=4, space="PSUM") as otp,
            tc.tile_pool(name="ptp", bufs=6) as ptp,
        ):
            ot_ps = [otp.tile([H + 1, 512], fp32, tag="ot", name=f"ot{j}") for j in range(TJ)]

            def emit_st(i):
                j0 = i // 4
                pts = {}
                for jj2 in range(i // 8, 2):
                    st = stp.tile([P, 1024], fp32, tag="st", name=f"st{i}_{jj2}")
                    pt = ptp.tile([P, 1024], bf16, tag="pt", name=f"pt{i}_{jj2}")
                    estart = None
                    for hh in range(2):
                        j = 2 * jj2 + hh
                        if j < j0:
                            continue
                        o = max(0, 128 * i - 512 * j)
                        lo = 512 * hh + o
                        nc.tensor.matmul(
                            st[:, lo : 512 * (hh + 1)], qt_sb[:, ts(i, P)],
                            kt_sb[:, 512 * j + o : 512 * (j + 1)],
                            start=True, stop=True,
                        )
                        if estart is None:
                            estart = lo
                    nc.scalar.activation(pt[:, estart:1024], st[:, estart:1024], EXP)
                    if jj2 == i // 8:
                        dlo = 128 * (i % 8)
                        nc.vector.tensor_tensor(
                            pt[:, dlo : dlo + P], pt[:, dlo : dlo + P], tri[:],
                            mybir.AluOpType.mult,
                        )
                    pts[jj2] = pt
                return pts

            def emit_pv(i, pts):
                j0 = i // 4
                for j in range(j0, TJ):
                    o = max(0, 128 * i - 512 * j)
                    pt = pts[j // 2]
                    lo = 512 * (j % 2) + o
                    nc.tensor.matmul(
                        ot_ps[j][:, o:512], vaug_sb[:, i, :],
                        pt[:, lo : 512 * (j % 2) + 512],
                        start=(i == 0), stop=(i == 4 * j + 3),
                    )

            prev = None
            for i in range(NT):
                pts = emit_st(i)
                if prev is not None:
                    emit_pv(prev[0], prev[1])
                prev = (i, pts)
            emit_pv(prev[0], prev[1])

            for j in range(TJ):
                nc.vector.tensor_copy(ot_sb[:, ts(j, 512)], ot_ps[j])

        with tc.tile_pool(name="orp", bufs=2, space="PSUM") as orp:
            for i in range(NT):
                ops = orp.tile([P, H + 1], fp32, tag="or", name=f"or{i}")
                nc.tensor.transpose(ops, ot_sb[:, ts(i, P)], ident[0 : H + 1, 0 : H + 1])
                nc.vector.reciprocal(rec_sb[:, i : i + 1], ops[:, H : H + 1])
                nc.any.tensor_scalar_mul(o_sb[:, i, :], ops[:, 0:H], rec_sb[:, i : i + 1])

        nc.sync.dma_start(out_d.rearrange("(i p) d -> p i d", p=P), o_sb[:])

    return nc


def _split_multiwaits(nc):
    """Walrus codegen only supports one sync-wait command per instruction;
    hoist extra waits onto NoOps inserted just before (same engine queue,
    identical semantics since engines execute their queue in order)."""
    import concourse.mybir as mybir

    n = 0
    for fn in nc.m.functions:
        for block in fn.blocks:
            new_insts = []
            for inst in block.instructions:
                si = inst.sync_info
                if si is not None and si.on_wait and len(si.on_wait) > 1:
                    waits = list(si.on_wait)
                    for w in waits[:-1]:
                        n += 1
                        new_insts.append(
                            mybir.InstNoOp(
                                name=f"WH-{n}", engine=inst.engine, ins=[], outs=[],
                                sync_info=mybir.SyncInfo(on_wait=[w], on_update=[]),
                            )
                        )
                    si.on_wait = waits[-1:]
                new_insts.append(inst)
            block.instructions = new_insts
    return nc


def _get_nc():
    if "nc" not in _NC_CACHE:
        _NC_CACHE["nc"] = _split_multiwaits(_build_nc())
    return _NC_CACHE["nc"]


def _make_in_maps(x, Wk, Wq, Wv):
    import ml_dtypes

    bf16 = ml_dtypes.bfloat16
    scale = 1.0 / np.sqrt(np.float32(C))
    wkq = np.ascontiguousarray(
        np.concatenate([Wk * scale, Wq], axis=0).T.astype(bf16)
    )  # [C, 128]
    wv = np.ascontiguousarray(Wv.T.astype(bf16))  # [C, 64]
    in_maps = []
    for b in range(B):
        xt = np.ascontiguousarray(x[b].T.astype(bf16))  # [C, T]
        in_maps.append({"xt": xt, "wkq": wkq, "wv": wv})
    return in_maps


def run(x, Wk, Wq, Wv, trace=False):
    from concourse.bass_utils import run_bass_kernel_spmd

    nc = _get_nc()
    in_maps = _make_in_maps(x, Wk, Wq, Wv)
    res = run_bass_kernel_spmd(nc, in_maps, core_ids=list(range(N_CORES)), trace=trace)
    out = np.stack([np.asarray(res.results[b]["out"]) for b in range(B)], axis=0)
    return out.astype(np.float32), res


def kernel(x, Wk, Wq, Wv):
    out, _ = run(x, Wk, Wq, Wv, trace=False)
    return out



# revision 8
# speedup vs baseline: 1.1249x; 1.1249x over previous
"""Trainium2 Bass kernel for nn_Head (single-head causal attention).

Contract: kernel(**inputs) takes FULL inputs (x [8,2048,1024] f32,
Wk/Wq/Wv [64,1024] f32) and returns the FULL output [8,2048,64] f32.
Data-parallel over batch B=8 across the 8 NeuronCores (one batch row per
core); each core runs an identical single-core program.

v2 design (strip-pipelined, row-tiled):
  - Host sends xT [C,T] bf16 + packed weights wts[2][C,128]:
      wts[0] = [Wk/32 ; Wq]  -> kq proj: kT @ partitions 0:64, qT @ 64:128
      wts[1] = [Wv ; Wk/32]  -> vk proj: vT @ partitions 0:64, kT2 @ 64:128
    The second kT copy is free: the v projection would otherwise leave
    half the PE array columns idle (M=64).
  - Work is pipelined over 4 column strips of 512 t-columns:
      kq proj strip j -> vk proj strip j (+ v transposes) -> ST pieces
      (i,j) for s-tiles i<=4j+3 as row-tiled pairs (two K=64 matmuls run
      concurrently in the upper/lower PE halves via partition placement)
      -> PSUM drains to SBUF bf16 with the causal mask folded in
      ((st*1) + (-1e9 on masked diag cols), so exp gives exact zeros)
      -> ONE exp activation per strip (multi-piece AP, amortizes the
      ~352-cycle ACTIVATE overhead) -> PV strip j-1 (row-tiled K=64
      pairs into OT_A/OT_B PSUM banks; ones-column gives the softmax
      denominator) -> epilogue: OT_A+OT_B, PE transpose, normalize,
      per-strip DMA out.
  - Row tiling operand placement:
      A-side (tile (0,0)):   q even tiles + kT   @ partitions 0:64
      B-side (tile (64,0)):  q odd tiles  + kT2  @ partitions 64:128
    q tiles live at partitions 64:128 after the kq projection; a small
    SBUF->SBUF DMA per strip duplicates them to 0:64 for the A side.
  - HAM warm-up: the PE re-throttles to 1.2 GHz after idle windows and
    only returns to 2.4 GHz after ~3.4us of sustained activity.  Dummy
    matmuls on scratch data run while the input DMA lands so real work
    starts warm.
  - A primer activation at the head of the scalar queue pulls the
    ~1.3us exp-table load into the DMA shadow.
"""

import sys

if "/opt/trn_rl_repo" not in sys.path:
    sys.path.insert(0, "/opt/trn_rl_repo")

import numpy as np

B = 8
T = 2048
C = 1024
H = 64
P = 128
CB = C // P        # 8 contraction chunks of 128
TJ = T // 512      # 4 column strips of 512
NT = T // P        # 16 s-tiles
N_CORES = 8

PIECES = [4 * j + 4 for j in range(TJ)]   # ST pieces per strip: 4, 8, 12, 16
N_DUMMY = 10                              # PE warm-up matmuls (N=512)
NEG = -1.0e9                              # causal mask fill

_NC_CACHE = {}


def _build_nc():
    import concourse.bass as bass
    import concourse.mybir as mybir
    import concourse.tile as tile
    from concourse.bass import ts
    from concourse.masks import make_identity

    fp32 = mybir.dt.float32
    bf16 = mybir.dt.bfloat16
    EXP = mybir.ActivationFunctionType.Exp
    MULT = mybir.AluOpType.mult
    ADD = mybir.AluOpType.add

    nc = bass.Bass(target_bir_lowering=False, debug=False)
    xt_d = nc.declare_dram_parameter("xt", [C, T], bf16, isOutput=False)
    wts_d = nc.declare_dram_parameter("wts", [2, C, P], bf16, isOutput=False)
    out_d = nc.declare_dram_parameter("out", [T, H], fp32, isOutput=True)

    from contextlib import ExitStack

    with tile.TileContext(nc) as tc, ExitStack() as stk:
        pers = stk.enter_context(tc.tile_pool(name="pers", bufs=1))
        # xt strip tiles (strip 0 split in halves so compute starts early);
        # one tile per DMA => exact dependency granularity.
        xt0a = pers.tile([P, CB // 2, 512], bf16, tag="xt0a", name="xt0a")
        xt0b = pers.tile([P, CB // 2, 512], bf16, tag="xt0b", name="xt0b")
        xts = [None] + [
            pers.tile([P, CB, 512], bf16, tag=f"xt{j}", name=f"xt{j}")
            for j in range(1, TJ)
        ]
        w_sb = pers.tile([P, 2, CB, P], bf16, tag="w_sb", name="w_sb")
        kqt = [pers.tile([P, 512], bf16, tag=f"kqt{j}", name=f"kqt{j}") for j in range(TJ)]
        vk = [pers.tile([P, 512], bf16, tag=f"vk{j}", name=f"vk{j}") for j in range(TJ)]
        qdup = [pers.tile([P, 512], bf16, tag=f"qd{j}", name=f"qd{j}") for j in range(TJ)]
        st_sb = [pers.tile([P, PIECES[j], 512], bf16, tag=f"st{j}", name=f"st{j}") for j in range(TJ)]
        pt_sb = [pers.tile([P, PIECES[j], 512], bf16, tag=f"pt{j}", name=f"pt{j}") for j in range(TJ)]
        vaug = [pers.tile([P, 4, H + 1], bf16, tag=f"va{j}", name=f"va{j}") for j in range(TJ)]
        oadd = [pers.tile([H + 1, 512], fp32, tag=f"oa{j}", name=f"oa{j}") for j in range(TJ)]
        o_sb = [pers.tile([P, 4, H], fp32, tag=f"o{j}", name=f"o{j}") for j in range(TJ)]
        rec = [pers.tile([P, 4], fp32, tag=f"rc{j}", name=f"rc{j}") for j in range(TJ)]
        ident = pers.tile([H + 1, H + 1], fp32, tag="ident", name="ident")
        identb = pers.tile([H, H], bf16, tag="identb", name="identb")
        # trineg[s, t] = 0 where t >= s (keep), NEG where t < s (mask)
        trineg = pers.tile([P, P], fp32, tag="trineg", name="trineg")
        scr_w = pers.tile([P, P], bf16, tag="scr_w", name="scr_w")
        scr_x = pers.tile([P, 512], bf16, tag="scr_x", name="scr_x")
        prim = pers.tile([P, 1], fp32, tag="prim", name="prim")
        prim_o = pers.tile([P, 1], fp32, tag="prim_o", name="prim_o")

        # ---- early gpsimd work: scratch memsets, masks, identities ----
        nc.gpsimd.memset(scr_w[:], 0.0)
        nc.gpsimd.memset(scr_x[:], 0.0)
        nc.gpsimd.memset(prim[:], 0.0)
        make_identity(nc, ident[:])
        make_identity(nc, identb[:])
        nc.gpsimd.memset(trineg[:], 0.0)
        # out[x, y] = (-x + y) >= 0 ? in : NEG   (keep y >= x)
        nc.gpsimd.affine_select(
            out=trineg[:], in_=trineg[:],
            compare_op=mybir.AluOpType.is_ge,
            fill=NEG, base=0, pattern=[[1, P]], channel_multiplier=-1,
        )
        for j in range(TJ):
            nc.gpsimd.memset(vaug[j][:, :, H], 1.0)

        # ---- scalar primer: pull the exp table load into the DMA shadow ----
        nc.scalar.activation(prim_o[:], prim[:], EXP)

        # ---- input DMAs (sync queue, priority order) ----
        nc.sync.dma_start(w_sb[:], wts_d.rearrange("w (cb p) m -> p w cb m", p=P))
        src = xt_d.rearrange("(cb p) t -> p cb t", p=P)
        nc.sync.dma_start(xt0a[:], src[:, 0 : CB // 2, 0:512])
        nc.sync.dma_start(xt0b[:], src[:, CB // 2 : CB, 0:512])
        for j in range(1, TJ):
            nc.sync.dma_start(xts[j][:], src[:, :, ts(j, 512)])

        def xsrc(j, cb):
            if j == 0:
                half = xt0a if cb < CB // 2 else xt0b
                return half[:, cb % (CB // 2), :]
            return xts[j][:, cb, :]

        # ---- PE warm-up dummies ----
        with tc.tile_pool(name="scrp", bufs=1, space="PSUM") as scrp:
            scr_ps = scrp.tile([P, 512], fp32, tag="scr", name="scr_ps")
            for _ in range(N_DUMMY):
                nc.tensor.matmul(scr_ps, scr_w[:], scr_x[:], start=True, stop=True)

        with (
            tc.tile_pool(name="prjp", bufs=2, space="PSUM") as prjp,   # 2 banks
            tc.tile_pool(name="stp", bufs=2, space="PSUM") as stp,     # 2 banks
            tc.tile_pool(name="otp", bufs=1, space="PSUM") as otp,     # 4 banks
        ):
            ot_a = otp.tile([H + 1, 512], fp32, tag="ota", name="ot_a")
            ot_b = otp.tile([H + 1, 512], fp32, tag="otb", name="ot_b")

            def emit_kqp(j):
                kq_ps = prjp.tile([P, 512], fp32, tag="prj", name=f"kq{j}")
                for cb in range(CB):
                    nc.tensor.matmul(
                        kq_ps, w_sb[:, 0, cb, :], xsrc(j, cb),
                        start=(cb == 0), stop=(cb == CB - 1),
                    )
                nc.vector.tensor_copy(kqt[j][:], kq_ps)
                # duplicate this strip's q tiles to partitions 0:64 (A-side)
                nc.sync.dma_start(qdup[j][0:H, :], kqt[j][H:P, :])

            def emit_vkp(j):
                v_ps = prjp.tile([P, 512], fp32, tag="prj", name=f"v{j}")
                for cb in range(CB):
                    nc.tensor.matmul(
                        v_ps, w_sb[:, 1, cb, :], xsrc(j, cb),
                        start=(cb == 0), stop=(cb == CB - 1),
                    )
                nc.vector.tensor_copy(vk[j][:], v_ps)
                # v transposes: vk[j][0:64, 128-block m] -> vaug[j][:, m, 0:64]
                for m in range(4):
                    vps = otp.tile([P, H], bf16, tag="vt", name=f"vt{j}_{m}")
                    nc.tensor.transpose(vps, vk[j][0:H, ts(m, P)], identb[:])
                    nc.vector.tensor_copy(vaug[j][:, m, 0:H], vps)

            def drain_piece(eng, j, i, ps, o):
                """PSUM piece (i,j) cols [o:512] -> st_sb, folding the causal
                mask into the diagonal 128-block."""
                if 4 * j <= i <= 4 * j + 3:        # diagonal piece
                    eng.scalar_tensor_tensor(
                        st_sb[j][:, i, o : o + P], ps[:, o : o + P], 1.0,
                        trineg[:], MULT, ADD,
                    )
                    if o + P < 512:
                        eng.tensor_copy(st_sb[j][:, i, o + P : 512], ps[:, o + P : 512])
                else:
                    eng.tensor_copy(st_sb[j][:, i, o:512], ps[:, o:512])

            def emit_st(j):
                # row-tiled pairs: tile 2m (A, partitions 0:64) runs
                # concurrently with tile 2m+1 (B, partitions 64:128)
                for m in range(2 * j + 2):
                    iA, iB = 2 * m, 2 * m + 1
                    oA = max(0, P * iA - 512 * j)
                    oB = max(0, P * iB - 512 * j)
                    sA = stp.tile([P, 512], fp32, tag="st", name=f"sa{j}_{m}")
                    sB = stp.tile([P, 512], fp32, tag="st", name=f"sb{j}_{m}")
                    nc.tensor.matmul(
                        sA[:, oA:512],
                        qdup[iA // 4][0:H, ts(iA % 4, P)],
                        kqt[j][0:H, oA:512],
                        start=True, stop=True,
                    )
                    nc.tensor.matmul(
                        sB[:, oB:512],
                        kqt[iB // 4][H:P, ts(iB % 4, P)],
                        vk[j][H:P, oB:512],
                        start=True, stop=True,
                    )
                    drain_piece(nc.vector, j, iA, sA, oA)
                    drain_piece(nc.vector, j, iB, sB, oB)

            def emit_exp(j, lo, hi):
                nc.scalar.activation(
                    pt_sb[j][:, lo:hi, :], st_sb[j][:, lo:hi, :], EXP
                )

            def emit_pv(j, lo, hi):
                n = PIECES[j]
                for i in range(lo, hi):
                    o = max(0, P * i - 512 * j)
                    nc.tensor.matmul(
                        ot_a[:, o:512],
                        vaug[i // 4][0:H, i % 4, :],
                        pt_sb[j][0:H, i, o:512],
                        start=(i == 0), stop=(i == n - 1),
                    )
                    nc.tensor.matmul(
                        ot_b[:, o:512],
                        vaug[i // 4][H:P, i % 4, :],
                        pt_sb[j][H:P, i, o:512],
                        start=(i == 0), stop=(i == n - 1),
                    )

            def emit_epilogue(j):
                # OT = A + B (only one PSUM operand allowed per DVE op)
                nc.vector.tensor_copy(oadd[j][:], ot_a[:, :])
                nc.vector.tensor_tensor(oadd[j][:], ot_b[:, :], oadd[j][:], ADD)
                for m in range(4):
                    ops = otp.tile([P, H + 1], fp32, tag="or", name=f"or{j}_{m}")
                    nc.tensor.transpose(ops, oadd[j][:, ts(m, P)], ident[:])
                    nc.vector.reciprocal(rec[j][:, m : m + 1], ops[:, H : H + 1])
                    nc.vector.tensor_scalar_mul(
                        o_sb[j][:, m, :], ops[:, 0:H], rec[j][:, m : m + 1]
                    )
                nc.sync.dma_start(
                    out_d[ts(j, 512), :].rearrange("(m p) d -> p m d", p=P),
                    o_sb[j][:],
                )

            # ---- the pipeline: PV lags one strip behind ST/exp ----
            for j in range(TJ):
                emit_kqp(j)
                emit_vkp(j)
                emit_st(j)
                if j < TJ - 1:
                    emit_exp(j, 0, PIECES[j])
                else:
                    emit_exp(j, 0, PIECES[j] // 2)      # split the last strip
                    emit_exp(j, PIECES[j] // 2, PIECES[j])
                if j > 0:
                    emit_pv(j - 1, 0, PIECES[j - 1])
                    emit_epilogue(j - 1)
            j = TJ - 1
            emit_pv(j, 0, PIECES[j] // 2)
            emit_pv(j, PIECES[j] // 2, PIECES[j])
            emit_epilogue(j)

    return nc


def _split_multiwaits(nc):
    """Walrus codegen only supports one sync-wait command per instruction;
    hoist extra waits onto NoOps inserted just before (same engine queue,
    identical semantics since engines execute their queue in order)."""
    import concourse.mybir as mybir

    n = 0
    for fn in nc.m.functions:
        for block in fn.blocks:
            new_insts = []
            for inst in block.instructions:
                si = inst.sync_info
                if si is not None and si.on_wait and len(si.on_wait) > 1:
                    waits = list(si.on_wait)
                    for w in waits[:-1]:
                        n += 1
                        new_insts.append(
                            mybir.InstNoOp(
                                name=f"WH-{n}", engine=inst.engine, ins=[], outs=[],
                                sync_info=mybir.SyncInfo(on_wait=[w], on_update=[]),
                            )
                        )
                    si.on_wait = waits[-1:]
                new_insts.append(inst)
            block.instructions = new_insts
    return nc


def _get_nc():
    if "nc" not in _NC_CACHE:
        _NC_CACHE["nc"] = _split_multiwaits(_build_nc())
    return _NC_CACHE["nc"]


def _make_in_maps(x, Wk, Wq, Wv):
    import ml_dtypes

    bf16 = ml_dtypes.bfloat16
    scale = 1.0 / np.sqrt(np.float32(C))
    wkq = np.concatenate([Wk * scale, Wq], axis=0).T  # [C, 128]
    wvk = np.concatenate([Wv, Wk * scale], axis=0).T  # [C, 128]
    wts = np.ascontiguousarray(np.stack([wkq, wvk]).astype(bf16))
    in_maps = []
    for b in range(B):
        xt = np.ascontiguousarray(x[b].T.astype(bf16))  # [C, T]
        in_maps.append({"xt": xt, "wts": wts})
    return in_maps


def run(x, Wk, Wq, Wv, trace=False):
    from concourse.bass_utils import run_bass_kernel_spmd

    nc = _get_nc()
    in_maps = _make_in_maps(x, Wk, Wq, Wv)
    res = run_bass_kernel_spmd(nc, in_maps, core_ids=list(range(N_CORES)), trace=trace)
    out = np.stack([np.asarray(res.results[b]["out"]) for b in range(B)], axis=0)
    return out.astype(np.float32), res


def kernel(x, Wk, Wq, Wv):
    out, _ = run(x, Wk, Wq, Wv, trace=False)
    return out


# revision 10
# speedup vs baseline: 1.2709x; 1.1297x over previous
"""Trainium2 Bass kernel for nn_Head (single-head causal attention).

Contract: kernel(**inputs) takes FULL inputs (x [8,2048,1024] f32,
Wk/Wq/Wv [64,1024] f32) and returns the FULL output [8,2048,64] f32.
Data-parallel over batch B=8 across the 8 NeuronCores (one batch row per
core); each core runs an identical single-core program.

v3 design (strip-pipelined, row-tiled, PSUM-direct exp):
  - Host sends xT [C,T] bf16 + packed weights wts[2][C,128]:
      wts[0] = [Wk/32 ; Wq]  -> kq proj: kT @ partitions 0:64, qT @ 64:128
      wts[1] = [Wv ; Wk/32]  -> vk proj: vT @ partitions 0:64, kT2 @ 64:128
    The second kT copy is free: the v projection would otherwise leave
    half the PE array columns idle (M=64).
  - Work is pipelined over 4 column strips of 512 t-columns.  Per strip:
    kq proj -> vk proj (+ v transposes) -> ST pieces as row-tiled pairs
    (tile 2m on PE rows 0:63 runs concurrently with tile 2m+1 on rows
    64:127) landing in [128,2,512] two-bank PSUM groups -> ONE exp per
    pair DIRECTLY from PSUM (ACT reads PSUM at full rate; DVE reads it
    at ~half rate, so this kills ~27us of DVE drains) -> tri-mask of
    diagonal blocks on DVE (bf16, cheap) -> PV strip (row-tiled K=64
    pairs into OT_A/OT_B banks; the ones-column in vaug gives the
    softmax denominator) -> epilogue: OT_A+OT_B -> SBUF, PE transposes,
    reciprocal on DVE, normalize multiplies on GpSimd (SBUF-only),
    per-strip DMA out.  PV lags exp by one strip.
  - Row tiling operand placement:
      A-side (tile (0,0)):   q even tiles + kT   @ partitions 0:64
      B-side (tile (64,0)):  q odd tiles  + kT2  @ partitions 64:128
    q tiles land at partitions 64:128 after the kq projection; a small
    SBUF->SBUF DMA per strip duplicates them to 0:64 for the A side.
  - GPSIMD cannot access PSUM on TRN2: it only does init work, the
    SBUF-side normalize, and qdup/output DMA descriptor issue.
  - HAM warm-up: the PE re-throttles to 1.2 GHz after idle windows and
    re-warms only after ~3.4us sustained activity.  A few dummy matmuls
    run while the input DMA lands so real work starts warm; optional
    scratch LDWEIGHTS fillers keep the array active in ACT-paced gaps.
  - A primer activation pulls the ~1.3us exp-table load into the DMA
    shadow.
"""

import sys

if "/opt/trn_rl_repo" not in sys.path:
    sys.path.insert(0, "/opt/trn_rl_repo")

import numpy as np

B = 8
T = 2048
C = 1024
H = 64
P = 128
CB = C // P        # 8 contraction chunks of 128
TJ = T // 512      # 4 column strips of 512
NT = T // P        # 16 s-tiles
N_CORES = 8

PIECES = [4 * j + 4 for j in range(TJ)]   # ST pieces per strip: 4, 8, 12, 16
N_DUMMY = 5                               # PE warm-up matmuls (N=512)
N_KW = 0                                  # scratch ldweights per PV pair

_NC_CACHE = {}


def _build_nc():
    import concourse.bass as bass
    import concourse.mybir as mybir
    import concourse.tile as tile
    from concourse.bass import ts
    from concourse.masks import make_identity, make_upper_triangular

    fp32 = mybir.dt.float32
    bf16 = mybir.dt.bfloat16
    EXP = mybir.ActivationFunctionType.Exp
    MULT = mybir.AluOpType.mult
    ADD = mybir.AluOpType.add

    nc = bass.Bass(target_bir_lowering=False, debug=False)
    xt_d = nc.declare_dram_parameter("xt", [C, T], bf16, isOutput=False)
    wts_d = nc.declare_dram_parameter("wts", [2, C, P], bf16, isOutput=False)
    out_d = nc.declare_dram_parameter("out", [T, H], fp32, isOutput=True)

    from contextlib import ExitStack

    with tile.TileContext(nc) as tc, ExitStack() as stk:
        pers = stk.enter_context(tc.tile_pool(name="pers", bufs=1))
        # xt strip tiles (strip 0 split in halves so compute starts early);
        # one tile per DMA => exact dependency granularity.
        xt0a = pers.tile([P, CB // 2, 512], bf16, tag="xt0a", name="xt0a")
        xt0b = pers.tile([P, CB // 2, 512], bf16, tag="xt0b", name="xt0b")
        xts = [None] + [
            pers.tile([P, CB, 512], bf16, tag=f"xt{j}", name=f"xt{j}")
            for j in range(1, TJ)
        ]
        w_sb = pers.tile([P, 2, CB, P], bf16, tag="w_sb", name="w_sb")
        kqt = [pers.tile([P, 512], bf16, tag=f"kqt{j}", name=f"kqt{j}") for j in range(TJ)]
        vk = [pers.tile([P, 512], bf16, tag=f"vk{j}", name=f"vk{j}") for j in range(TJ)]
        qdup = [pers.tile([P, 512], bf16, tag=f"qd{j}", name=f"qd{j}") for j in range(TJ)]
        pt_sb = [pers.tile([P, PIECES[j], 512], bf16, tag=f"pt{j}", name=f"pt{j}") for j in range(TJ)]
        vaug = [pers.tile([P, 4, H + 1], bf16, tag=f"va{j}", name=f"va{j}") for j in range(TJ)]
        oadd = [pers.tile([H + 1, 512], fp32, tag=f"oa{j}", name=f"oa{j}") for j in range(TJ)]
        otr = [pers.tile([P, 4, H + 1], fp32, tag=f"otr{j}", name=f"otr{j}") for j in range(TJ)]
        o_sb = [pers.tile([P, 4, H], fp32, tag=f"o{j}", name=f"o{j}") for j in range(TJ)]
        rec = [pers.tile([P, 4], fp32, tag=f"rc{j}", name=f"rc{j}") for j in range(TJ)]
        ident = pers.tile([H + 1, H + 1], fp32, tag="ident", name="ident")
        identb = pers.tile([H, H], bf16, tag="identb", name="identb")
        tri = pers.tile([P, P], bf16, tag="tri", name="tri")
        scr_w = pers.tile([P, P], bf16, tag="scr_w", name="scr_w")
        scr_x = pers.tile([P, 512], bf16, tag="scr_x", name="scr_x")
        prim = pers.tile([P, 1], fp32, tag="prim", name="prim")
        prim_o = pers.tile([P, 1], fp32, tag="prim_o", name="prim_o")

        # ---- early gpsimd work: scratch memsets, masks, identities ----
        nc.gpsimd.memset(scr_w[:], 0.0)
        nc.gpsimd.memset(scr_x[:], 0.0)
        nc.gpsimd.memset(prim[:], 0.0)
        make_identity(nc, ident[:])
        make_identity(nc, identb[:])
        # tri[s, t] = 1 where t >= s else 0 (upper triangular incl diagonal)
        make_upper_triangular(nc, tri[:], val=1.0, diag=True)
        for j in range(TJ):
            nc.gpsimd.memset(vaug[j][:, :, H], 1.0)

        # ---- input DMAs: weights on sync, strip 0 on scalar (parallel
        # issue), remaining strips on sync ----
        nc.sync.dma_start(w_sb[:], wts_d.rearrange("w (cb p) m -> p w cb m", p=P))
        src = xt_d.rearrange("(cb p) t -> p cb t", p=P)
        nc.scalar.dma_start(xt0a[:], src[:, 0 : CB // 2, 0:512])
        nc.scalar.dma_start(xt0b[:], src[:, CB // 2 : CB, 0:512])

        # ---- scalar primer: pull the exp table load into the DMA shadow ----
        nc.scalar.activation(prim_o[:], prim[:], EXP)
        for j in range(1, TJ):
            nc.sync.dma_start(xts[j][:], src[:, :, ts(j, 512)])

        def xsrc(j, cb):
            if j == 0:
                half = xt0a if cb < CB // 2 else xt0b
                return half[:, cb % (CB // 2), :]
            return xts[j][:, cb, :]

        # ---- PE warm-up dummies ----
        with tc.tile_pool(name="scrp", bufs=1, space="PSUM") as scrp:
            scr_ps = scrp.tile([P, 512], fp32, tag="scr", name="scr_ps")
            for _ in range(N_DUMMY):
                nc.tensor.matmul(scr_ps, scr_w[:], scr_x[:], start=True, stop=True)

        with (
            tc.tile_pool(name="prjp", bufs=2, space="PSUM") as prjp,   # 2 banks
            tc.tile_pool(name="stp", bufs=2, space="PSUM") as stp,     # 4 banks
            tc.tile_pool(name="otp", bufs=1, space="PSUM") as otp,     # 2 banks
        ):
            ot_a = otp.tile([H + 1, 512], fp32, tag="ota", name="ot_a")
            ot_b = otp.tile([H + 1, 512], fp32, tag="otb", name="ot_b")

            def emit_kqp(j):
                kq_ps = prjp.tile([P, 512], fp32, tag="prj", name=f"kq{j}")
                for cb in range(CB):
                    nc.tensor.matmul(
                        kq_ps, w_sb[:, 0, cb, :], xsrc(j, cb),
                        start=(cb == 0), stop=(cb == CB - 1),
                    )
                nc.vector.tensor_copy(kqt[j][:], kq_ps)
                # duplicate this strip's q tiles to partitions 0:64 (A-side
                # weights); SBUF->SBUF DMA issued from the gpsimd queue.
                nc.gpsimd.dma_start(qdup[j][0:H, :], kqt[j][H:P, :])

            def emit_vkp(j):
                v_ps = prjp.tile([P, 512], fp32, tag="prj", name=f"v{j}")
                for cb in range(CB):
                    nc.tensor.matmul(
                        v_ps, w_sb[:, 1, cb, :], xsrc(j, cb),
                        start=(cb == 0), stop=(cb == CB - 1),
                    )
                nc.vector.tensor_copy(vk[j][:], v_ps)
                # v transposes: vk[j][0:64, 128-block m] -> vaug[j][:, m, 0:64]
                for m in range(4):
                    vps = prjp.tile([P, H], bf16, tag="prj", name=f"vt{j}_{m}")
                    nc.tensor.transpose(vps, vk[j][0:H, ts(m, P)], identb[:])
                    nc.vector.tensor_copy(vaug[j][:, m, 0:H], vps)

            def emit_st(j):
                # row-tiled pairs: tile 2m (A, partitions 0:64) runs
                # concurrently with tile 2m+1 (B, partitions 64:128).
                # Each pair lands in one [128,2,512] two-bank PSUM group;
                # exp reads the group directly from PSUM.
                for m in range(2 * j + 2):
                    iA, iB = 2 * m, 2 * m + 1
                    oA = max(0, P * iA - 512 * j)
                    oB = max(0, P * iB - 512 * j)
                    s2 = stp.tile([P, 2, 512], fp32, tag="st", name=f"s{j}_{m}")
                    nc.tensor.matmul(
                        s2[:, 1, oB:512],
                        kqt[iB // 4][H:P, ts(iB % 4, P)],
                        vk[j][H:P, oB:512],
                        start=True, stop=True,
                    )
                    nc.tensor.matmul(
                        s2[:, 0, oA:512],
                        qdup[iA // 4][0:H, ts(iA % 4, P)],
                        kqt[j][0:H, oA:512],
                        start=True, stop=True,
                    )
                    nc.scalar.activation(
                        pt_sb[j][:, 2 * m : 2 * m + 2, oA:512],
                        s2[:, :, oA:512],
                        EXP,
                    )
                    # tri-mask the diagonal blocks (post-exp, bf16, cheap)
                    if 4 * j <= iA <= 4 * j + 3:
                        nc.vector.tensor_tensor(
                            pt_sb[j][:, iA, oA : oA + P],
                            pt_sb[j][:, iA, oA : oA + P],
                            tri[:], MULT,
                        )
                    if 4 * j <= iB <= 4 * j + 3:
                        nc.vector.tensor_tensor(
                            pt_sb[j][:, iB, oB : oB + P],
                            pt_sb[j][:, iB, oB : oB + P],
                            tri[:], MULT,
                        )

            def emit_pv(j):
                n = PIECES[j]
                for i in range(n):
                    o = max(0, P * i - 512 * j)
                    nc.tensor.matmul(
                        ot_a[:, o:512],
                        vaug[i // 4][0:H, i % 4, :],
                        pt_sb[j][0:H, i, o:512],
                        start=(i == 0), stop=(i == n - 1),
                    )
                    nc.tensor.matmul(
                        ot_b[:, o:512],
                        vaug[i // 4][H:P, i % 4, :],
                        pt_sb[j][H:P, i, o:512],
                        start=(i == 0), stop=(i == n - 1),
                    )
                    for _ in range(N_KW):
                        nc.tensor.ldweights(scr_w[0:H, :])

            def emit_epilogue(j):
                # OT = A + B into SBUF (DVE; one PSUM operand per op)
                nc.vector.tensor_copy(oadd[j][:], ot_a[:, :])
                nc.vector.tensor_tensor(oadd[j][:], ot_b[:, :], oadd[j][:], ADD)
                for m in range(4):
                    ops = prjp.tile([P, H + 1], fp32, tag="prj", name=f"or{j}_{m}")
                    nc.tensor.transpose(ops, oadd[j][:, ts(m, P)], ident[:])
                    nc.vector.tensor_copy(otr[j][:, m, :], ops)
                # reciprocal of the 4 denominator columns at once (SBUF)
                nc.vector.reciprocal(rec[j][:, :], otr[j][:, :, H])
                # normalize on gpsimd (SBUF-only engine), then DMA out
                for m in range(4):
                    nc.gpsimd.tensor_scalar_mul(
                        o_sb[j][:, m, :], otr[j][:, m, 0:H], rec[j][:, m : m + 1]
                    )
                nc.gpsimd.dma_start(
                    out_d[ts(j, 512), :].rearrange("(m p) d -> p m d", p=P),
                    o_sb[j][:],
                )

            # ---- the pipeline: PV lags exp by one strip ----
            for j in range(TJ):
                emit_kqp(j)
                emit_vkp(j)
                emit_st(j)
                if j > 0:
                    emit_pv(j - 1)
                    emit_epilogue(j - 1)
            emit_pv(TJ - 1)
            emit_epilogue(TJ - 1)

    return nc


def _split_multiwaits(nc):
    """Walrus codegen only supports one sync-wait command per instruction;
    hoist extra waits onto NoOps inserted just before (same engine queue,
    identical semantics since engines execute their queue in order)."""
    import concourse.mybir as mybir

    n = 0
    for fn in nc.m.functions:
        for block in fn.blocks:
            new_insts = []
            for inst in block.instructions:
                si = inst.sync_info
                if si is not None and si.on_wait and len(si.on_wait) > 1:
                    waits = list(si.on_wait)
                    for w in waits[:-1]:
                        n += 1
                        new_insts.append(
                            mybir.InstNoOp(
                                name=f"WH-{n}", engine=inst.engine, ins=[], outs=[],
                                sync_info=mybir.SyncInfo(on_wait=[w], on_update=[]),
                            )
                        )
                    si.on_wait = waits[-1:]
                new_insts.append(inst)
            block.instructions = new_insts
    return nc


def _get_nc():
    if "nc" not in _NC_CACHE:
        _NC_CACHE["nc"] = _split_multiwaits(_build_nc())
    return _NC_CACHE["nc"]


def _make_in_maps(x, Wk, Wq, Wv):
    import ml_dtypes

    bf16 = ml_dtypes.bfloat16
    scale = 1.0 / np.sqrt(np.float32(C))
    wkq = np.concatenate([Wk * scale, Wq], axis=0).T  # [C, 128]
    wvk = np.concatenate([Wv, Wk * scale], axis=0).T  # [C, 128]
    wts = np.ascontiguousarray(np.stack([wkq, wvk]).astype(bf16))
    in_maps = []
    for b in range(B):
        xt = np.ascontiguousarray(x[b].T.astype(bf16))  # [C, T]
        in_maps.append({"xt": xt, "wts": wts})
    return in_maps


def run(x, Wk, Wq, Wv, trace=False):
    from concourse.bass_utils import run_bass_kernel_spmd

    nc = _get_nc()
    in_maps = _make_in_maps(x, Wk, Wq, Wv)
    res = run_bass_kernel_spmd(nc, in_maps, core_ids=list(range(N_CORES)), trace=trace)
    out = np.stack([np.asarray(res.results[b]["out"]) for b in range(B)], axis=0)
    return out.astype(np.float32), res


def kernel(x, Wk, Wq, Wv):
    out, _ = run(x, Wk, Wq, Wv, trace=False)
    return out


# revision 12
# speedup vs baseline: 1.2947x; 1.0188x over previous
"""Trainium2 Bass kernel for nn_Head (single-head causal attention).

Contract: kernel(**inputs) takes FULL inputs (x [8,2048,1024] f32,
Wk/Wq/Wv [64,1024] f32) and returns the FULL output [8,2048,64] f32.
Data-parallel over batch B=8 across the 8 NeuronCores (one batch row per
core); each core runs an identical single-core program.

v3 design (strip-pipelined, row-tiled, PSUM-direct exp):
  - Host sends xT [C,T] bf16 + packed weights wts[2][C,128]:
      wts[0] = [Wk/32 ; Wq]  -> kq proj: kT @ partitions 0:64, qT @ 64:128
      wts[1] = [Wv ; Wk/32]  -> vk proj: vT @ partitions 0:64, kT2 @ 64:128
    The second kT copy is free: the v projection would otherwise leave
    half the PE array columns idle (M=64).
  - Work is pipelined over 4 column strips of 512 t-columns.  Per strip:
    kq proj -> vk proj (+ v transposes) -> ST pieces as row-tiled pairs
    (tile 2m on PE rows 0:63 runs concurrently with tile 2m+1 on rows
    64:127) landing in [128,2,512] two-bank PSUM groups -> ONE exp per
    pair DIRECTLY from PSUM (ACT reads PSUM at full rate; DVE reads it
    at ~half rate, so this kills ~27us of DVE drains) -> tri-mask of
    diagonal blocks on DVE (bf16, cheap) -> PV strip (row-tiled K=64
    pairs into OT_A/OT_B banks; the ones-column in vaug gives the
    softmax denominator) -> epilogue: OT_A+OT_B -> SBUF, PE transposes,
    reciprocal on DVE, normalize multiplies on GpSimd (SBUF-only),
    per-strip DMA out.  PV lags exp by one strip.
  - Row tiling operand placement:
      A-side (tile (0,0)):   q even tiles + kT   @ partitions 0:64
      B-side (tile (64,0)):  q odd tiles  + kT2  @ partitions 64:128
    q tiles land at partitions 64:128 after the kq projection; a small
    SBUF->SBUF DMA per strip duplicates them to 0:64 for the A side.
  - GPSIMD cannot access PSUM on TRN2: it only does init work, the
    SBUF-side normalize, and qdup/output DMA descriptor issue.
  - HAM warm-up: the PE re-throttles to 1.2 GHz after idle windows and
    re-warms only after ~3.4us sustained activity.  A few dummy matmuls
    run while the input DMA lands so real work starts warm; optional
    scratch LDWEIGHTS fillers keep the array active in ACT-paced gaps.
  - A primer activation pulls the ~1.3us exp-table load into the DMA
    shadow.
"""

import sys

if "/opt/trn_rl_repo" not in sys.path:
    sys.path.insert(0, "/opt/trn_rl_repo")

import numpy as np

B = 8
T = 2048
C = 1024
H = 64
P = 128
CB = C // P        # 8 contraction chunks of 128
TJ = T // 512      # 4 column strips of 512
NT = T // P        # 16 s-tiles
N_CORES = 8

PIECES = [4 * j + 4 for j in range(TJ)]   # ST pieces per strip: 4, 8, 12, 16
N_DUMMY = 5                               # PE warm-up matmuls (N=512)
N_KW = 0                                  # scratch ldweights per PV pair

_NC_CACHE = {}


def _build_nc():
    import concourse.bass as bass
    import concourse.mybir as mybir
    import concourse.tile as tile
    from concourse.bass import ts
    from concourse.masks import make_identity, make_upper_triangular

    fp32 = mybir.dt.float32
    bf16 = mybir.dt.bfloat16
    EXP = mybir.ActivationFunctionType.Exp
    MULT = mybir.AluOpType.mult
    ADD = mybir.AluOpType.add

    nc = bass.Bass(target_bir_lowering=False, debug=False)
    xt_d = nc.declare_dram_parameter("xt", [C, T], bf16, isOutput=False)
    wts_d = nc.declare_dram_parameter("wts", [2, C, P], bf16, isOutput=False)
    out_d = nc.declare_dram_parameter("out", [T, H], fp32, isOutput=True)

    from contextlib import ExitStack

    with tile.TileContext(nc) as tc, ExitStack() as stk:
        pers = stk.enter_context(tc.tile_pool(name="pers", bufs=1))
        # xt strip tiles (strip 0 split in halves so compute starts early);
        # one tile per DMA => exact dependency granularity.
        xt0a = pers.tile([P, CB // 2, 512], bf16, tag="xt0a", name="xt0a")
        xt0b = pers.tile([P, CB // 2, 512], bf16, tag="xt0b", name="xt0b")
        xts = [None] + [
            pers.tile([P, CB, 512], bf16, tag=f"xt{j}", name=f"xt{j}")
            for j in range(1, TJ)
        ]
        w_sb = pers.tile([P, 2, CB, P], bf16, tag="w_sb", name="w_sb")
        kqt = [pers.tile([P, 512], bf16, tag=f"kqt{j}", name=f"kqt{j}") for j in range(TJ)]
        kk2 = [pers.tile([P, 512], bf16, tag=f"kk2{j}", name=f"kk2{j}") for j in range(TJ)]
        vk = [pers.tile([P, 512], bf16, tag=f"vk{j}", name=f"vk{j}") for j in range(TJ)]
        qdup = [pers.tile([P, 512], bf16, tag=f"qd{j}", name=f"qd{j}") for j in range(TJ)]
        pt_sb = [pers.tile([P, PIECES[j], 512], bf16, tag=f"pt{j}", name=f"pt{j}") for j in range(TJ)]
        vaug = [pers.tile([P, 4, H + 1], bf16, tag=f"va{j}", name=f"va{j}") for j in range(TJ)]
        oadd = [pers.tile([H + 1, 512], fp32, tag=f"oa{j}", name=f"oa{j}") for j in range(TJ)]
        o_sb = [pers.tile([P, 4, H], fp32, tag=f"o{j}", name=f"o{j}") for j in range(TJ)]
        rec = [pers.tile([P, 4], fp32, tag=f"rc{j}", name=f"rc{j}") for j in range(TJ)]
        ident = pers.tile([H + 1, H + 1], fp32, tag="ident", name="ident")
        identb = pers.tile([H, H], bf16, tag="identb", name="identb")
        tri = pers.tile([P, P], bf16, tag="tri", name="tri")
        scr_w = pers.tile([P, P], bf16, tag="scr_w", name="scr_w")
        scr_x = pers.tile([P, 512], bf16, tag="scr_x", name="scr_x")
        prim = pers.tile([P, 1], fp32, tag="prim", name="prim")
        prim_o = pers.tile([P, 1], fp32, tag="prim_o", name="prim_o")

        # ---- early gpsimd work: scratch memsets, masks, identities ----
        nc.gpsimd.memset(scr_w[:], 0.0)
        nc.gpsimd.memset(scr_x[:], 0.0)
        nc.gpsimd.memset(prim[:], 0.0)
        make_identity(nc, ident[:])
        make_identity(nc, identb[:])
        # tri[s, t] = 1 where t >= s else 0 (upper triangular incl diagonal)
        make_upper_triangular(nc, tri[:], val=1.0, diag=True)
        for j in range(TJ):
            nc.gpsimd.memset(vaug[j][:, :, H], 1.0)

        # ---- input DMAs: ALL on one ring (sync) so transfers execute
        # strictly in strip order at full bandwidth ----
        nc.sync.dma_start(w_sb[:], wts_d.rearrange("w (cb p) m -> p w cb m", p=P))
        src = xt_d.rearrange("(cb p) t -> p cb t", p=P)
        nc.sync.dma_start(xt0a[:], src[:, 0 : CB // 2, 0:512])
        nc.sync.dma_start(xt0b[:], src[:, CB // 2 : CB, 0:512])
        for j in range(1, TJ):
            nc.sync.dma_start(xts[j][:], src[:, :, ts(j, 512)])

        # ---- scalar primer: pull the exp table load into the DMA shadow ----
        nc.scalar.activation(prim_o[:], prim[:], EXP)

        def xsrc(j, cb):
            if j == 0:
                half = xt0a if cb < CB // 2 else xt0b
                return half[:, cb % (CB // 2), :]
            return xts[j][:, cb, :]

        # ---- PE warm-up dummies ----
        with tc.tile_pool(name="scrp", bufs=1, space="PSUM") as scrp:
            scr_ps = scrp.tile([P, 512], fp32, tag="scr", name="scr_ps")
            for _ in range(N_DUMMY):
                nc.tensor.matmul(scr_ps, scr_w[:], scr_x[:], start=True, stop=True)

        with (
            tc.tile_pool(name="prjp", bufs=2, space="PSUM") as prjp,   # 2 banks
            tc.tile_pool(name="stp", bufs=2, space="PSUM") as stp,     # 4 banks
            tc.tile_pool(name="otp", bufs=1, space="PSUM") as otp,     # 2 banks
        ):
            ot_a = otp.tile([H + 1, 512], fp32, tag="ota", name="ot_a")
            ot_b = otp.tile([H + 1, 512], fp32, tag="otb", name="ot_b")

            def emit_kqp(j):
                kq_ps = prjp.tile([P, 512], fp32, tag="prj", name=f"kq{j}")
                for cb in range(CB):
                    nc.tensor.matmul(
                        kq_ps, w_sb[:, 0, cb, :], xsrc(j, cb),
                        start=(cb == 0), stop=(cb == CB - 1),
                    )
                # kT -> kk2 top half (A-side rhs); qT -> kqt bottom
                # (B-side weights).  kk2 bottom gets kT2 from the vk proj so
                # both row-tile rhs streams read the SAME tile/columns
                # (required for lockstep row-tile concurrency).
                nc.vector.tensor_copy(kk2[j][0:H, :], kq_ps[0:H, :])
                nc.vector.tensor_copy(kqt[j][H:P, :], kq_ps[H:P, :])
                # duplicate this strip's q tiles to partitions 0:64 (A-side
                # weights); SBUF->SBUF DMA issued from the gpsimd queue.
                nc.gpsimd.dma_start(qdup[j][0:H, :], kqt[j][H:P, :])

            def emit_vkp(j):
                v_ps = prjp.tile([P, 512], fp32, tag="prj", name=f"v{j}")
                for cb in range(CB):
                    nc.tensor.matmul(
                        v_ps, w_sb[:, 1, cb, :], xsrc(j, cb),
                        start=(cb == 0), stop=(cb == CB - 1),
                    )
                nc.vector.tensor_copy(vk[j][0:H, :], v_ps[0:H, :])
                nc.vector.tensor_copy(kk2[j][H:P, :], v_ps[H:P, :])
                # v transposes: vk[j][0:64, 128-block m] -> vaug[j][:, m, 0:64]
                for m in range(4):
                    vps = prjp.tile([P, H], bf16, tag="prj", name=f"vt{j}_{m}")
                    nc.tensor.transpose(vps, vk[j][0:H, ts(m, P)], identb[:])
                    nc.vector.tensor_copy(vaug[j][:, m, 0:H], vps)

            def emit_st(j):
                # row-tiled pairs: tile 2m (A, partitions 0:64) runs
                # concurrently with tile 2m+1 (B, partitions 64:128).
                # Each pair lands in one [128,2,512] two-bank PSUM group;
                # exp reads the group directly from PSUM.
                for m in range(2 * j + 2):
                    iA, iB = 2 * m, 2 * m + 1
                    o = max(0, P * iA - 512 * j)   # shared column start
                    s2 = stp.tile([P, 2, 512], fp32, tag="st", name=f"s{j}_{m}")
                    nc.tensor.matmul(
                        s2[:, 0, o:512],
                        qdup[iA // 4][0:H, ts(iA % 4, P)],
                        kk2[j][0:H, o:512],
                        start=True, stop=True,
                    )
                    nc.tensor.matmul(
                        s2[:, 1, o:512],
                        kqt[iB // 4][H:P, ts(iB % 4, P)],
                        kk2[j][H:P, o:512],
                        start=True, stop=True,
                    )
                    nc.scalar.activation(
                        pt_sb[j][:, 2 * m : 2 * m + 2, o:512],
                        s2[:, :, o:512],
                        EXP,
                    )
                    yield m

            def emit_tri(j):
                # tri-mask the 4 diagonal pieces of strip j (before PV j)
                for i in range(4 * j, 4 * j + 4):
                    o = P * i - 512 * j
                    nc.vector.tensor_tensor(
                        pt_sb[j][:, i, o : o + P],
                        pt_sb[j][:, i, o : o + P],
                        tri[:], MULT,
                    )

            def emit_pv(j, lo=0, hi=None):
                n = PIECES[j]
                if hi is None:
                    hi = n
                for i in range(lo, hi):
                    o = max(0, P * i - 512 * j)
                    nc.tensor.matmul(
                        ot_a[:, o:512],
                        vaug[i // 4][0:H, i % 4, :],
                        pt_sb[j][0:H, i, o:512],
                        start=(i == 0), stop=(i == n - 1),
                    )
                    nc.tensor.matmul(
                        ot_b[:, o:512],
                        vaug[i // 4][H:P, i % 4, :],
                        pt_sb[j][H:P, i, o:512],
                        start=(i == 0), stop=(i == n - 1),
                    )
                    for _ in range(N_KW):
                        nc.tensor.ldweights(scr_w[0:H, :])

            def emit_epilogue(j):
                # OT = A + B into SBUF (DVE; one PSUM operand per op)
                nc.vector.tensor_copy(oadd[j][:], ot_a[:, :])
                nc.vector.tensor_tensor(oadd[j][:], ot_b[:, :], oadd[j][:], ADD)
                for m in range(4):
                    ops = prjp.tile([P, H + 1], fp32, tag="prj", name=f"or{j}_{m}")
                    nc.tensor.transpose(ops, oadd[j][:, ts(m, P)], ident[:])
                    nc.vector.reciprocal(rec[j][:, m : m + 1], ops[:, H : H + 1])
                    nc.vector.tensor_scalar_mul(
                        o_sb[j][:, m, :], ops[:, 0:H], rec[j][:, m : m + 1]
                    )
                nc.gpsimd.dma_start(
                    out_d[ts(j, 512), :].rearrange("(m p) d -> p m d", p=P),
                    o_sb[j][:],
                )

            # ---- the pipeline: PV lags exp by one strip, its chunks
            # interleaved between the next strip's ST pairs so the PE has
            # work queued while the ACT catches up ----
            for j in range(TJ):
                emit_kqp(j)
                emit_vkp(j)
                if j > 0:
                    emit_tri(j - 1)
                npv = PIECES[j - 1] if j > 0 else 0
                done = 0
                nst = 2 * j + 2
                for m in emit_st(j):
                    if j > 0:
                        take = (npv * (m + 1)) // nst
                        emit_pv(j - 1, done, take)
                        done = take
                if j > 0:
                    emit_pv(j - 1, done, npv)
                    emit_epilogue(j - 1)
            emit_tri(TJ - 1)
            emit_pv(TJ - 1)
            emit_epilogue(TJ - 1)

    return nc


def _split_multiwaits(nc):
    """Walrus codegen only supports one sync-wait command per instruction;
    hoist extra waits onto NoOps inserted just before (same engine queue,
    identical semantics since engines execute their queue in order)."""
    import concourse.mybir as mybir

    n = 0
    for fn in nc.m.functions:
        for block in fn.blocks:
            new_insts = []
            for inst in block.instructions:
                si = inst.sync_info
                if si is not None and si.on_wait and len(si.on_wait) > 1:
                    waits = list(si.on_wait)
                    for w in waits[:-1]:
                        n += 1
                        new_insts.append(
                            mybir.InstNoOp(
                                name=f"WH-{n}", engine=inst.engine, ins=[], outs=[],
                                sync_info=mybir.SyncInfo(on_wait=[w], on_update=[]),
                            )
                        )
                    si.on_wait = waits[-1:]
                new_insts.append(inst)
            block.instructions = new_insts
    return nc


def _get_nc():
    if "nc" not in _NC_CACHE:
        _NC_CACHE["nc"] = _split_multiwaits(_build_nc())
    return _NC_CACHE["nc"]


def _make_in_maps(x, Wk, Wq, Wv):
    import ml_dtypes

    bf16 = ml_dtypes.bfloat16
    scale = 1.0 / np.sqrt(np.float32(C))
    wkq = np.concatenate([Wk * scale, Wq], axis=0).T  # [C, 128]
    wvk = np.concatenate([Wv, Wk * scale], axis=0).T  # [C, 128]
    wts = np.ascontiguousarray(np.stack([wkq, wvk]).astype(bf16))
    in_maps = []
    for b in range(B):
        xt = np.ascontiguousarray(x[b].T.astype(bf16))  # [C, T]
        in_maps.append({"xt": xt, "wts": wts})
    return in_maps


def run(x, Wk, Wq, Wv, trace=False):
    from concourse.bass_utils import run_bass_kernel_spmd

    nc = _get_nc()
    in_maps = _make_in_maps(x, Wk, Wq, Wv)
    res = run_bass_kernel_spmd(nc, in_maps, core_ids=list(range(N_CORES)), trace=trace)
    out = np.stack([np.asarray(res.results[b]["out"]) for b in range(B)], axis=0)
    return out.astype(np.float32), res


def kernel(x, Wk, Wq, Wv):
    out, _ = run(x, Wk, Wq, Wv, trace=False)
    return out
